# revision 1
# baseline (speedup 1.0000x reference)
# Causal self-attention kernel for 8 Trainium2 NeuronCores.
#
# Problem (hardcoded): B=2, S=2048, D=1024, H=16 heads of dk=64.
#   q,k,v = x @ W.T + b (torch Linear), per-head causal softmax attention,
#   out[b,s,:] = concat_h(attn_h @ v_h). No output projection.
#
# Sharding: 8 cores = 2 batches x 4 head-groups. Core c handles batch c//4
# and heads [4*(c%4), 4*(c%4)+4) => output channels [256*(c%4), +256).
# No cross-device communication.
#
# Per-core design (engine-balance driven):
#   - ACT is the critical engine: softmax exp runs only there (~1 elem/cyc/
#     lane + ~352cyc fixed cost per ACTIVATE). Scores are built in big
#     [128, <=1024] PSUM segments so exp runs as few, wide calls as possible.
#   - All matmuls use float32r (full PE rate at moving-dim>=256, ~2e-4 rel
#     error). Weights pre-transposed/augmented on CPU (parameter packing).
#   - x is PE-transposed to xT (d on partitions); qT/kT [e,s] and v [s,e+1]
#     projections; v carries a ones column so the PV matmul also produces
#     softmax denominators (row 64 of the accumulator).
#   - Attention per (head, sq-half): key-block j outer; scoresT[sk,sq] on PE,
#     additive -1e9 causal mask on the diagonal block, one exp per segment,
#     PV accumulates outT_aug[65, 1024] in PSUM across j.
#   - Tail: copy accumulator to SBUF, PE-transpose back incl. sums row,
#     reciprocal + per-partition scale into the output staging tile.
#   - Emission interleaving: only head 0/1's first-half prerequisites are
#     emitted up front; the rest of the projections are drip-fed into the
#     attention phase ("filler"), filling PE while ACT grinds through exp.

import numpy as np

B, S, D, H = 2, 2048, 1024, 16
DK = D // H            # 64
NCORES = 8
HPC = 4                # heads per core
E = HPC * DK           # 256 output channels per core
EA = HPC * (DK + 1)    # 260 augmented v width (ones col per head)
P = 128
NSB = S // P           # 16 s-blocks
NDC = D // P           # 8 d-chunks
HALF = 1024

_cache = {}


def _build_module():
    import concourse.bacc as bacc
    import concourse.mybir as mybir
    import concourse.tile as tile

    f32 = mybir.dt.float32
    f32r = mybir.dt.float32r
    Exp = mybir.ActivationFunctionType.Exp
    Copy = mybir.ActivationFunctionType.Copy

    nc = bacc.Bacc("TRN2", target_bir_lowering=False, debug=False)

    x_d = nc.dram_tensor("x", [S, D], f32r, kind="ExternalInput")
    wq_d = nc.dram_tensor("wq_t", [D, E], f32r, kind="ExternalInput")
    wk_d = nc.dram_tensor("wk_t", [D, E], f32r, kind="ExternalInput")
    wv_d = nc.dram_tensor("wv_t", [D, EA], f32r, kind="ExternalInput")
    bq_d = nc.dram_tensor("bq", [1, E], f32, kind="ExternalInput")
    bk_d = nc.dram_tensor("bk", [1, E], f32, kind="ExternalInput")
    bv_d = nc.dram_tensor("bv", [1, EA], f32r, kind="ExternalInput")
    mask_d = nc.dram_tensor("mask", [P, P], f32r, kind="ExternalInput")
    ident_d = nc.dram_tensor("ident", [P, P], f32, kind="ExternalInput")
    ones_d = nc.dram_tensor("ones", [1, P], f32r, kind="ExternalInput")
    out_d = nc.dram_tensor("out", [S, E], f32, kind="ExternalOutput")

    with tile.TileContext(nc) as tc:
        with (
            tc.tile_pool(name="consts", bufs=1) as consts,
            tc.tile_pool(name="qkv", bufs=1) as qkv,
            tc.tile_pool(name="outst", bufs=1) as outst,
            tc.tile_pool(name="xin", bufs=4) as xin,
            tc.tile_pool(name="xt", bufs=1) as xtp,
            tc.tile_pool(name="pp", bufs=2, space="PSUM") as pp,
            tc.tile_pool(name="psc", bufs=2, space="PSUM") as pscp,
            tc.tile_pool(name="pacc", bufs=1, space="PSUM") as paccp,
            tc.tile_pool(name="attn", bufs=3) as attnp,
            tc.tile_pool(name="otn", bufs=2) as otnp,
        ):
            # ---- constants ----
            wq_sb = consts.tile([P, NDC, E], f32r, tag="wq")
            wk_sb = consts.tile([P, NDC, E], f32r, tag="wk")
            wv_sb = consts.tile([P, NDC, EA], f32r, tag="wv")
            bqc_sb = consts.tile([P, 2], f32, tag="bqc")
            bkc_sb = consts.tile([P, 2], f32, tag="bkc")
            bv_sb = consts.tile([1, EA], f32r, tag="bv")
            mask_sb = consts.tile([P, P], f32r, tag="mask")
            identr_sb = consts.tile([P, P], f32r, tag="identr")
            ident_sb = consts.tile([P, P], f32, tag="ident")
            ones_sb = consts.tile([1, P], f32r, tag="ones")

            x_tiles = {}

            def emit_x_dma(sb):
                x_tile = xin.tile([P, D], f32r, tag="x")
                nc.sync.dma_start(out=x_tile, in_=x_d[sb * P:(sb + 1) * P, :])
                x_tiles[sb] = x_tile

            for _sb in range(8):
                emit_x_dma(_sb)

            nc.sync.dma_start(out=wq_sb, in_=wq_d[:].rearrange("(c p) e -> p c e", p=P))
            nc.sync.dma_start(out=wk_sb, in_=wk_d[:].rearrange("(c p) e -> p c e", p=P))
            nc.sync.dma_start(out=wv_sb, in_=wv_d[:].rearrange("(c p) e -> p c e", p=P))
            nc.sync.dma_start(out=bqc_sb, in_=bq_d[:].rearrange("o (c p) -> p (o c)", p=P))
            nc.sync.dma_start(out=bkc_sb, in_=bk_d[:].rearrange("o (c p) -> p (o c)", p=P))
            nc.sync.dma_start(out=bv_sb, in_=bv_d[:])
            nc.sync.dma_start(out=mask_sb, in_=mask_d[:])
            nc.sync.dma_start(out=ident_sb, in_=ident_d[:])
            nc.sync.dma_start(out=identr_sb, in_=ident_d[:].bitcast(f32r))
            nc.sync.dma_start(out=ones_sb, in_=ones_d[:])

            qT = qkv.tile([P, 2, S], f32r, tag="qT")
            kT = qkv.tile([P, 2, S], f32r, tag="kT")
            v_sb = qkv.tile([P, NSB, EA], f32r, tag="v")
            out_sb = outst.tile([P, NSB, E], f32, tag="out")
            # xT: [d%128, s-block, d-chunk, 128] so transpose copies are
            # contiguous 512-wide (d-chunk groups of 4)
            xT = xtp.tile([P, NSB, NDC, P], f32r, tag="xT")

            nalt = [0]  # alternator for copy engine balance
            ACT_COPIES = False  # exp owns ACT; keep its queue clear

            def copy_balanced(dst, src, act_ok=True):
                if ACT_COPIES and act_ok and nalt[0] % 2 == 0:
                    nc.scalar.copy(dst, src)
                else:
                    nc.vector.tensor_copy(dst, src)
                nalt[0] += 1

            def emit_xt(sb, dcg, act_ok=True):
                # transpose 4 d-chunks of x block sb into one psum tile
                if dcg == 0 and sb not in x_tiles:
                    emit_x_dma(sb)
                ptile = pp.tile([P, 512], f32r, tag="pp")
                for k in range(4):
                    dc = dcg * 4 + k
                    nc.tensor.transpose(
                        ptile[:, k * P:(k + 1) * P],
                        x_tiles[sb][:, dc * P:(dc + 1) * P],
                        identr_sb,
                    )
                copy_balanced(xT[:, sb, dcg * 4:(dcg + 1) * 4, :], ptile, act_ok)

            def qk_rhs(dc, lo, w):
                # xT view for d-chunk dc, s columns [lo, lo+w) (128-aligned)
                sb0 = lo // P
                return xT[:, sb0:sb0 + w // P, dc, :]

            def emit_qk_proj(which, eb, sc, act_ok=True):
                w_sb = wq_sb if which == 0 else wk_sb
                bc = bqc_sb if which == 0 else bkc_sb
                dst = qT if which == 0 else kT
                ps = pp.tile([P, 512], f32, tag="pp")
                for dc in range(NDC):
                    nc.tensor.matmul(
                        ps,
                        lhsT=w_sb[:, dc, eb * P:(eb + 1) * P],
                        rhs=qk_rhs(dc, sc * 512, 512),
                        start=(dc == 0),
                        stop=(dc == NDC - 1),
                    )
                dst_ap = dst[:, eb, sc * 512:(sc + 1) * 512]
                nc.vector.tensor_scalar_add(dst_ap, ps, bc[:, eb:eb + 1])
                nalt[0] += 1

            def emit_v_proj(sb, act_ok=True):
                ps = pp.tile([P, 512], f32, tag="pp")
                pv = ps[:, :EA]
                for dc in range(NDC):
                    nc.tensor.matmul(
                        pv,
                        lhsT=xT[:, sb, dc, :],
                        rhs=wv_sb[:, dc, :],
                        start=(dc == 0),
                        stop=False,
                    )
                nc.tensor.matmul(
                    pv,
                    lhsT=ones_sb[0:1, :],
                    rhs=bv_sb[0:1, :],
                    start=False,
                    stop=True,
                )
                copy_balanced(v_sb[:, sb, :], pv, act_ok)

            # ---- phase A, sliced so attention starts early ----
            def phase_a_slice1():
                for sb in range(8):
                    emit_xt(sb, 0)
                    emit_xt(sb, 1)
                emit_qk_proj(0, 0, 0)
                emit_qk_proj(0, 0, 1)
                emit_qk_proj(1, 0, 0)
                for sb in (0, 1):
                    emit_v_proj(sb)

            filler = [lambda: emit_qk_proj(1, 0, 1, act_ok=False)]
            for sb in range(2, 8):
                filler.append(lambda sb=sb: emit_v_proj(sb, act_ok=False))
            for sc in (0, 1):
                for which in (0, 1):
                    filler.append(lambda which=which, sc=sc: emit_qk_proj(
                        which, 1, sc, act_ok=False))
            for sb in range(8, NSB):
                filler.append(lambda sb=sb: emit_xt(sb, 0, act_ok=False))
                filler.append(lambda sb=sb: emit_xt(sb, 1, act_ok=False))
            for sb in range(8, NSB):
                filler.append(lambda sb=sb: emit_v_proj(sb, act_ok=False))
            for sc in (2, 3):
                for which in (0, 1):
                    filler.append(lambda which=which, sc=sc: emit_qk_proj(
                        which, 0, sc, act_ok=False))
            for sc in (2, 3):
                for which in (0, 1):
                    filler.append(lambda which=which, sc=sc: emit_qk_proj(
                        which, 1, sc, act_ok=False))

            def drain(n):
                for _ in range(n):
                    if filler:
                        filler.pop(0)()

            # ---- attention ----
            def attn_head_half(h, half, jhook=None, per_block_dma=False):
                po = 64 * (h % 2)
                eb = h // 2
                kT_h = kT[po:po + DK, eb, :]
                qT_h = qT[po:po + DK, eb, :]
                lo = half * HALF
                hi = lo + HALF
                pacc = paccp.tile([65, HALF], f32, tag="pacc")

                def emit_pv(j, at):
                    # PV pieces: absolute 512-aligned within [lo, hi)
                    sb0 = max(j * P, lo)
                    lhsT_v = v_sb[:, j, h * 65:(h + 1) * 65]
                    m = sb0
                    while m < hi:
                        w = min(512 - (m - lo) % 512, hi - m)
                        bank = (m - lo) // 512
                        j_last = min((lo + 512 * (bank + 1)) // P - 1, hi // P - 1)
                        nc.tensor.matmul(
                            pacc[:, m - lo:m - lo + w],
                            lhsT=lhsT_v,
                            rhs=at[:, m - sb0:m - sb0 + w],
                            start=(j == 0),
                            stop=(j == j_last),
                        )
                        m += w

                pending = None  # (j, at) whose PV is deferred one iteration
                for j in range(hi // P):
                    ko = j * P
                    sb0 = max(ko, lo)
                    segw = hi - sb0
                    ps = pscp.tile([P, HALF], f32, tag="sc")
                    lhsT_k = kT_h[:, ko:ko + P]
                    m = 0
                    while m < segw:
                        w = min(512, segw - m)
                        if w < 256 and sb0 + m + 256 <= hi:
                            w = 256  # pad narrow f32r pieces to full rate
                        nc.tensor.matmul(
                            ps[:, m:m + w],
                            lhsT=lhsT_k,
                            rhs=qT_h[:, sb0 + m:sb0 + m + w],
                            start=True,
                            stop=True,
                        )
                        m += w
                    at = attnp.tile([P, HALF], f32r, tag="at")
                    nc.scalar.activation(
                        out=at[:, :segw], in_=ps[:, :segw], func=Exp, scale=0.125
                    )
                    if ko >= lo:
                        nc.gpsimd.tensor_mul(at[:, 0:P], at[:, 0:P], mask_sb)
                    if jhook is not None:
                        jhook(j)
                    if pending is not None:
                        emit_pv(*pending)
                    pending = (j, at)
                emit_pv(*pending)

                # tail: normalize + transpose back + stage
                otn = otnp.tile([65, HALF], f32, tag="otn")
                nc.vector.tensor_copy(otn, pacc)
                for il in range(HALF // P):
                    i = half * 8 + il
                    pot = pp.tile([P, 65], f32, tag="pp")
                    nc.tensor.transpose(
                        pot, otn[:, il * P:(il + 1) * P], ident_sb[0:65, 0:65]
                    )
                    linv = otnp.tile([P, 1], f32, tag="linv")
                    nc.vector.reciprocal(linv, pot[:, DK:DK + 1])
                    nc.vector.tensor_scalar_mul(
                        out_sb[:, i, h * DK:(h + 1) * DK], pot[:, 0:DK], linv
                    )
                    if per_block_dma:
                        nc.sync.dma_start(
                            out=out_d[i * P:(i + 1) * P, :], in_=out_sb[:, i, :]
                        )

            phase_a_slice1()
            hook = lambda j: drain(2)
            # prereq positions: (0,1)/(1,1) need everything through eb0-sc23
            # (34 units); (2,*) need eb1 (+8). 16 j-slots before (0,1) at
            # 2/j = 32; force-drain the remainder at each boundary.
            attn_head_half(0, 0, jhook=hook)
            attn_head_half(1, 0, jhook=hook)
            attn_head_half(2, 0, jhook=hook)
            attn_head_half(3, 0, jhook=hook)
            for i in range(8):
                nc.sync.dma_start(
                    out=out_d[i * P:(i + 1) * P, :], in_=out_sb[:, i, :]
                )
            drain(len(filler))
            attn_head_half(0, 1)
            attn_head_half(1, 1)
            attn_head_half(2, 1)
            attn_head_half(3, 1, per_block_dma=True)

    nc.compile()
    return nc


def _prep_core_inputs(inputs, c):
    x = np.ascontiguousarray(np.asarray(inputs["x"], dtype=np.float32))
    b, hg = c // HPC, c % HPC
    e0 = hg * E

    wq = np.asarray(inputs["Wq"], dtype=np.float32)
    wk = np.asarray(inputs["Wk"], dtype=np.float32)
    wv = np.asarray(inputs["Wv"], dtype=np.float32)
    bq = np.asarray(inputs["bq"], dtype=np.float32)
    bk = np.asarray(inputs["bk"], dtype=np.float32)
    bv = np.asarray(inputs["bv"], dtype=np.float32)

    wq_t = np.ascontiguousarray(wq[e0:e0 + E, :].T)          # [D, E]
    wk_t = np.ascontiguousarray(wk[e0:e0 + E, :].T)
    wv_t = np.zeros((D, EA), dtype=np.float32)
    bv_a = np.zeros((1, EA), dtype=np.float32)
    for lh in range(HPC):
        cols = slice(lh * 65, lh * 65 + DK)
        rows = slice(e0 + lh * DK, e0 + lh * DK + DK)
        wv_t[:, cols] = wv[rows, :].T
        bv_a[0, cols] = bv[rows]
        bv_a[0, lh * 65 + DK] = 1.0                          # ones column

    mask = np.where(
        np.arange(P)[None, :] >= np.arange(P)[:, None], 1.0, 0.0
    ).astype(np.float32)

    return {
        "x": np.ascontiguousarray(x[b]),
        "wq_t": wq_t,
        "wk_t": wk_t,
        "wv_t": wv_t,
        "bq": np.ascontiguousarray(bq[e0:e0 + E])[None, :],
        "bk": np.ascontiguousarray(bk[e0:e0 + E])[None, :],
        "bv": bv_a,
        "mask": mask,
        "ident": np.eye(P, dtype=np.float32),
        "ones": np.ones((1, P), dtype=np.float32),
    }


def kernel(**inputs):
    from concourse.bass_utils import run_bass_kernel_spmd

    if "nc" not in _cache:
        _cache["nc"] = _build_module()
    nc = _cache["nc"]

    in_maps = [_prep_core_inputs(inputs, c) for c in range(NCORES)]
    res = run_bass_kernel_spmd(nc, in_maps, core_ids=list(range(NCORES)))

    out = np.empty((B, S, D), dtype=np.float32)
    for c in range(NCORES):
        b, hg = c // HPC, c % HPC
        out[b, :, hg * E:(hg + 1) * E] = res.results[c]["out"]
    return out



# revision 3
# speedup vs baseline: 5.5783x; 5.5783x over previous
# Causal self-attention kernel for 8 Trainium2 NeuronCores.
#
# Problem (hardcoded): B=2, S=2048, D=1024, H=16 heads of dk=64.
#   q,k,v = x @ W.T + b (torch Linear), per-head causal softmax attention,
#   out[b,s,:] = concat_h(attn_h @ v_h). No output projection.
#
# Sharding: 8 cores = 2 batches x 4 head-groups. Core c handles batch c//4
# and heads [4*(c%4), 4*(c%4)+4) => output channels [256*(c%4), +256).
# No cross-device communication.
#
# Per-core design (engine-balance driven):
#   - ACT is the critical engine: softmax exp runs only there (~1 elem/cyc/
#     lane + ~352cyc fixed cost per ACTIVATE). Scores are built in big
#     [128, <=1024] PSUM segments so exp runs as few, wide calls as possible.
#   - All matmuls use float32r (full PE rate at moving-dim>=256, ~2e-4 rel
#     error). Weights pre-transposed/augmented on CPU (parameter packing).
#   - x is PE-transposed to xT (d on partitions); qT/kT [e,s] and v [s,e+1]
#     projections; v carries a ones column so the PV matmul also produces
#     softmax denominators (row 64 of the accumulator).
#   - Attention per (head, sq-half): key-block j outer; scoresT[sk,sq] on PE,
#     additive -1e9 causal mask on the diagonal block, one exp per segment,
#     PV accumulates outT_aug[65, 1024] in PSUM across j.
#   - Tail: copy accumulator to SBUF, PE-transpose back incl. sums row,
#     reciprocal + per-partition scale into the output staging tile.
#   - Emission interleaving: only head 0/1's first-half prerequisites are
#     emitted up front; the rest of the projections are drip-fed into the
#     attention phase ("filler"), filling PE while ACT grinds through exp.

import numpy as np

B, S, D, H = 2, 2048, 1024, 16
DK = D // H            # 64
NCORES = 8
HPC = 4                # heads per core
E = HPC * DK           # 256 output channels per core
EA = HPC * (DK + 1)    # 260 augmented v width (ones col per head)
P = 128
NSB = S // P           # 16 s-blocks
NDC = D // P           # 8 d-chunks
HALF = 1024

_cache = {}


def _build_module(reps=1):
    import concourse.bacc as bacc
    import concourse.mybir as mybir
    import concourse.tile as tile

    f32 = mybir.dt.float32
    f32r = mybir.dt.float32r
    Exp = mybir.ActivationFunctionType.Exp
    Copy = mybir.ActivationFunctionType.Copy

    nc = bacc.Bacc("TRN2", target_bir_lowering=False, debug=False)

    x_d = nc.dram_tensor("x", [S, D], f32r, kind="ExternalInput")
    wq_d = nc.dram_tensor("wq_t", [D, E], f32r, kind="ExternalInput")
    wk_d = nc.dram_tensor("wk_t", [D, E], f32r, kind="ExternalInput")
    wv_d = nc.dram_tensor("wv_t", [D, EA], f32r, kind="ExternalInput")
    bq_d = nc.dram_tensor("bq", [1, E], f32, kind="ExternalInput")
    bk_d = nc.dram_tensor("bk", [1, E], f32, kind="ExternalInput")
    bv_d = nc.dram_tensor("bv", [1, EA], f32r, kind="ExternalInput")
    mask_d = nc.dram_tensor("mask", [P, P], f32r, kind="ExternalInput")
    ident_d = nc.dram_tensor("ident", [P, P], f32, kind="ExternalInput")
    ones_d = nc.dram_tensor("ones", [1, P], f32r, kind="ExternalInput")
    out_d = nc.dram_tensor("out", [S, E], f32, kind="ExternalOutput")

    with tile.TileContext(nc) as tc:
        for rep in range(reps):
            _emit_rep(nc, tc, rep, x_d, wq_d, wk_d, wv_d, bq_d, bk_d, bv_d,
                      mask_d, ident_d, ones_d, out_d)

    nc.compile()
    return nc


def _emit_rep(nc, tc, rep, x_d, wq_d, wk_d, wv_d, bq_d, bk_d, bv_d,
              mask_d, ident_d, ones_d, out_d):
    import concourse.mybir as mybir
    import concourse.tile as tile

    f32 = mybir.dt.float32
    f32r = mybir.dt.float32r
    Exp = mybir.ActivationFunctionType.Exp

    if True:
        with (
            tc.tile_pool(name=f"consts{rep}", bufs=1) as consts,
            tc.tile_pool(name=f"qkv{rep}", bufs=1) as qkv,
            tc.tile_pool(name=f"outst{rep}", bufs=1) as outst,
            tc.tile_pool(name=f"xin{rep}", bufs=4) as xin,
            tc.tile_pool(name=f"xt{rep}", bufs=1) as xtp,
            tc.tile_pool(name=f"pp{rep}", bufs=2, space="PSUM") as pp,
            tc.tile_pool(name=f"psc{rep}", bufs=2, space="PSUM") as pscp,
            tc.tile_pool(name=f"pacc{rep}", bufs=1, space="PSUM") as paccp,
            tc.tile_pool(name=f"attn{rep}", bufs=3) as attnp,
            tc.tile_pool(name=f"otn{rep}", bufs=2) as otnp,
        ):
            # ---- constants ----
            wq_sb = consts.tile([P, NDC, E], f32r, tag="wq")
            wk_sb = consts.tile([P, NDC, E], f32r, tag="wk")
            wv_sb = consts.tile([P, NDC, EA], f32r, tag="wv")
            bqc_sb = consts.tile([P, 2], f32, tag="bqc")
            bkc_sb = consts.tile([P, 2], f32, tag="bkc")
            bv_sb = consts.tile([1, EA], f32r, tag="bv")
            mask_sb = consts.tile([P, P], f32r, tag="mask")
            identr_sb = consts.tile([P, P], f32r, tag="identr")
            ident_sb = consts.tile([P, P], f32, tag="ident")
            ones_sb = consts.tile([1, P], f32r, tag="ones")

            x_tiles = {}

            def emit_x_dma(sb):
                x_tile = xin.tile([P, D], f32r, tag="x")
                nc.sync.dma_start(out=x_tile, in_=x_d[sb * P:(sb + 1) * P, :])
                x_tiles[sb] = x_tile

            for _sb in range(8):
                emit_x_dma(_sb)

            nc.sync.dma_start(out=wq_sb, in_=wq_d[:].rearrange("(c p) e -> p c e", p=P))
            nc.sync.dma_start(out=wk_sb, in_=wk_d[:].rearrange("(c p) e -> p c e", p=P))
            nc.sync.dma_start(out=wv_sb, in_=wv_d[:].rearrange("(c p) e -> p c e", p=P))
            nc.sync.dma_start(out=bqc_sb, in_=bq_d[:].rearrange("o (c p) -> p (o c)", p=P))
            nc.sync.dma_start(out=bkc_sb, in_=bk_d[:].rearrange("o (c p) -> p (o c)", p=P))
            nc.sync.dma_start(out=bv_sb, in_=bv_d[:])
            nc.sync.dma_start(out=mask_sb, in_=mask_d[:])
            nc.sync.dma_start(out=ident_sb, in_=ident_d[:])
            nc.sync.dma_start(out=identr_sb, in_=ident_d[:].bitcast(f32r))
            nc.sync.dma_start(out=ones_sb, in_=ones_d[:])

            qT = qkv.tile([P, 2, S], f32r, tag="qT")
            kT = qkv.tile([P, 2, S], f32r, tag="kT")
            v_sb = qkv.tile([P, NSB, EA], f32r, tag="v")
            out_sb = outst.tile([P, NSB, E], f32, tag="out")
            # xT: [d%128, s-block, d-chunk, 128] so transpose copies are
            # contiguous 512-wide (d-chunk groups of 4)
            xT = xtp.tile([P, NSB, NDC, P], f32r, tag="xT")

            nalt = [0]  # alternator for copy engine balance
            ACT_COPIES = False  # exp owns ACT; keep its queue clear

            def copy_balanced(dst, src, act_ok=True):
                if ACT_COPIES and act_ok and nalt[0] % 2 == 0:
                    nc.scalar.copy(dst, src)
                else:
                    nc.vector.tensor_copy(dst, src)
                nalt[0] += 1

            def emit_xt(sb, dcg, act_ok=True):
                # transpose 4 d-chunks of x block sb into one psum tile
                if dcg == 0 and sb not in x_tiles:
                    emit_x_dma(sb)
                ptile = pp.tile([P, 512], f32r, tag="pp")
                for k in range(4):
                    dc = dcg * 4 + k
                    nc.tensor.transpose(
                        ptile[:, k * P:(k + 1) * P],
                        x_tiles[sb][:, dc * P:(dc + 1) * P],
                        identr_sb,
                    )
                copy_balanced(xT[:, sb, dcg * 4:(dcg + 1) * 4, :], ptile, act_ok)

            def qk_rhs(dc, lo, w):
                # xT view for d-chunk dc, s columns [lo, lo+w) (128-aligned)
                sb0 = lo // P
                return xT[:, sb0:sb0 + w // P, dc, :]

            def emit_qk_proj(which, eb, sc, act_ok=True):
                w_sb = wq_sb if which == 0 else wk_sb
                bc = bqc_sb if which == 0 else bkc_sb
                dst = qT if which == 0 else kT
                ps = pp.tile([P, 512], f32, tag="pp")
                for dc in range(NDC):
                    nc.tensor.matmul(
                        ps,
                        lhsT=w_sb[:, dc, eb * P:(eb + 1) * P],
                        rhs=qk_rhs(dc, sc * 512, 512),
                        start=(dc == 0),
                        stop=(dc == NDC - 1),
                    )
                dst_ap = dst[:, eb, sc * 512:(sc + 1) * 512]
                nc.vector.tensor_scalar_add(dst_ap, ps, bc[:, eb:eb + 1])
                nalt[0] += 1

            def emit_v_proj(sb, act_ok=True):
                ps = pp.tile([P, 512], f32, tag="pp")
                pv = ps[:, :EA]
                for dc in range(NDC):
                    nc.tensor.matmul(
                        pv,
                        lhsT=xT[:, sb, dc, :],
                        rhs=wv_sb[:, dc, :],
                        start=(dc == 0),
                        stop=False,
                    )
                nc.tensor.matmul(
                    pv,
                    lhsT=ones_sb[0:1, :],
                    rhs=bv_sb[0:1, :],
                    start=False,
                    stop=True,
                )
                copy_balanced(v_sb[:, sb, :], pv, act_ok)

            # ---- phase A, sliced so attention starts early ----
            def phase_a_slice1():
                for sb in range(8):
                    emit_xt(sb, 0)
                    emit_xt(sb, 1)
                emit_qk_proj(0, 0, 0)
                emit_qk_proj(0, 0, 1)
                emit_qk_proj(1, 0, 0)
                for sb in (0, 1):
                    emit_v_proj(sb)

            filler = [lambda: emit_qk_proj(1, 0, 1, act_ok=False)]
            for sb in range(2, 8):
                filler.append(lambda sb=sb: emit_v_proj(sb, act_ok=False))
            for sc in (0, 1):
                for which in (0, 1):
                    filler.append(lambda which=which, sc=sc: emit_qk_proj(
                        which, 1, sc, act_ok=False))
            for sb in range(8, NSB):
                filler.append(lambda sb=sb: emit_xt(sb, 0, act_ok=False))
                filler.append(lambda sb=sb: emit_xt(sb, 1, act_ok=False))
            for sb in range(8, NSB):
                filler.append(lambda sb=sb: emit_v_proj(sb, act_ok=False))
            for sc in (2, 3):
                for which in (0, 1):
                    filler.append(lambda which=which, sc=sc: emit_qk_proj(
                        which, 0, sc, act_ok=False))
            for sc in (2, 3):
                for which in (0, 1):
                    filler.append(lambda which=which, sc=sc: emit_qk_proj(
                        which, 1, sc, act_ok=False))

            def drain(n):
                for _ in range(n):
                    if filler:
                        filler.pop(0)()

            # ---- attention ----
            def attn_head_half(h, half, jhook=None, per_block_dma=False):
                po = 64 * (h % 2)
                eb = h // 2
                kT_h = kT[po:po + DK, eb, :]
                qT_h = qT[po:po + DK, eb, :]
                lo = half * HALF
                hi = lo + HALF
                pacc = paccp.tile([65, HALF], f32, tag="pacc")

                def emit_pv(j, at):
                    # PV pieces: absolute 512-aligned within [lo, hi)
                    sb0 = max(j * P, lo)
                    lhsT_v = v_sb[:, j, h * 65:(h + 1) * 65]
                    m = sb0
                    while m < hi:
                        w = min(512 - (m - lo) % 512, hi - m)
                        bank = (m - lo) // 512
                        j_last = min((lo + 512 * (bank + 1)) // P - 1, hi // P - 1)
                        nc.tensor.matmul(
                            pacc[:, m - lo:m - lo + w],
                            lhsT=lhsT_v,
                            rhs=at[:, m - sb0:m - sb0 + w],
                            start=(j == 0),
                            stop=(j == j_last),
                        )
                        m += w

                pending = None  # (j, at) whose PV is deferred one iteration
                for j in range(hi // P):
                    ko = j * P
                    sb0 = max(ko, lo)
                    segw = hi - sb0
                    ps = pscp.tile([P, HALF], f32, tag="sc")
                    lhsT_k = kT_h[:, ko:ko + P]
                    m = 0
                    while m < segw:
                        w = min(512, segw - m)
                        if w < 256 and sb0 + m + 256 <= hi:
                            w = 256  # pad narrow f32r pieces to full rate
                        nc.tensor.matmul(
                            ps[:, m:m + w],
                            lhsT=lhsT_k,
                            rhs=qT_h[:, sb0 + m:sb0 + m + w],
                            start=True,
                            stop=True,
                        )
                        m += w
                    at = attnp.tile([P, HALF], f32r, tag="at")
                    nc.scalar.activation(
                        out=at[:, :segw], in_=ps[:, :segw], func=Exp, scale=0.125
                    )
                    if ko >= lo:
                        nc.gpsimd.tensor_mul(at[:, 0:P], at[:, 0:P], mask_sb)
                    if jhook is not None:
                        jhook(j)
                    if pending is not None:
                        emit_pv(*pending)
                    pending = (j, at)
                emit_pv(*pending)

                # tail: normalize + transpose back + stage
                otn = otnp.tile([65, HALF], f32, tag="otn")
                nc.vector.tensor_copy(otn, pacc)
                for il in range(HALF // P):
                    i = half * 8 + il
                    pot = pp.tile([P, 65], f32, tag="pp")
                    nc.tensor.transpose(
                        pot, otn[:, il * P:(il + 1) * P], ident_sb[0:65, 0:65]
                    )
                    linv = otnp.tile([P, 1], f32, tag="linv")
                    nc.vector.reciprocal(linv, pot[:, DK:DK + 1])
                    nc.vector.tensor_scalar_mul(
                        out_sb[:, i, h * DK:(h + 1) * DK], pot[:, 0:DK], linv
                    )
                    if per_block_dma:
                        nc.sync.dma_start(
                            out=out_d[i * P:(i + 1) * P, :], in_=out_sb[:, i, :]
                        )

            phase_a_slice1()
            hook = lambda j: drain(2)
            # prereq positions: (0,1)/(1,1) need everything through eb0-sc23
            # (34 units); (2,*) need eb1 (+8). 16 j-slots before (0,1) at
            # 2/j = 32; force-drain the remainder at each boundary.
            attn_head_half(0, 0, jhook=hook)
            attn_head_half(1, 0, jhook=hook)
            attn_head_half(2, 0, jhook=hook)
            attn_head_half(3, 0, jhook=hook)
            for i in range(8):
                nc.sync.dma_start(
                    out=out_d[i * P:(i + 1) * P, :], in_=out_sb[:, i, :]
                )
            drain(len(filler))
            attn_head_half(0, 1)
            attn_head_half(1, 1)
            attn_head_half(2, 1)
            attn_head_half(3, 1, per_block_dma=True)


def _prep_core_inputs(inputs, c):
    x = np.ascontiguousarray(np.asarray(inputs["x"], dtype=np.float32))
    b, hg = c // HPC, c % HPC
    e0 = hg * E

    wq = np.asarray(inputs["Wq"], dtype=np.float32)
    wk = np.asarray(inputs["Wk"], dtype=np.float32)
    wv = np.asarray(inputs["Wv"], dtype=np.float32)
    bq = np.asarray(inputs["bq"], dtype=np.float32)
    bk = np.asarray(inputs["bk"], dtype=np.float32)
    bv = np.asarray(inputs["bv"], dtype=np.float32)

    wq_t = np.ascontiguousarray(wq[e0:e0 + E, :].T)          # [D, E]
    wk_t = np.ascontiguousarray(wk[e0:e0 + E, :].T)
    wv_t = np.zeros((D, EA), dtype=np.float32)
    bv_a = np.zeros((1, EA), dtype=np.float32)
    for lh in range(HPC):
        cols = slice(lh * 65, lh * 65 + DK)
        rows = slice(e0 + lh * DK, e0 + lh * DK + DK)
        wv_t[:, cols] = wv[rows, :].T
        bv_a[0, cols] = bv[rows]
        bv_a[0, lh * 65 + DK] = 1.0                          # ones column

    mask = np.where(
        np.arange(P)[None, :] >= np.arange(P)[:, None], 1.0, 0.0
    ).astype(np.float32)

    return {
        "x": np.ascontiguousarray(x[b]),
        "wq_t": wq_t,
        "wk_t": wk_t,
        "wv_t": wv_t,
        "bq": np.ascontiguousarray(bq[e0:e0 + E])[None, :],
        "bk": np.ascontiguousarray(bk[e0:e0 + E])[None, :],
        "bv": bv_a,
        "mask": mask,
        "ident": np.eye(P, dtype=np.float32),
        "ones": np.ones((1, P), dtype=np.float32),
    }


def kernel(**inputs):
    from concourse.bass_utils import run_bass_kernel_spmd

    if "nc" not in _cache:
        _cache["nc"] = _build_module()
    nc = _cache["nc"]

    in_maps = [_prep_core_inputs(inputs, c) for c in range(NCORES)]
    res = run_bass_kernel_spmd(nc, in_maps, core_ids=list(range(NCORES)))

    out = np.empty((B, S, D), dtype=np.float32)
    for c in range(NCORES):
        b, hg = c // HPC, c % HPC
        out[b, :, hg * E:(hg + 1) * E] = res.results[c]["out"]
    return out



# revision 5
# speedup vs baseline: 12.4249x; 2.2274x over previous
# Causal self-attention kernel for 8 Trainium2 NeuronCores — v2 (bf16).
#
# Problem (hardcoded): B=2, S=2048, D=1024, H=16 heads of dk=64.
#   q,k,v = x @ W.T + b (torch Linear), per-head causal softmax attention,
#   out[b,s,:] = concat_h(attn_h @ v_h). No output projection.
#
# Sharding: 8 cores = 2 batches x 4 head-groups. Core c handles batch c//4
# and heads [4*(c%4), 4*(c%4)+4) => output channels [256*(c%4), +256).
# No cross-device communication.
#
# Design (vs the original f32r kernel):
#   - bf16 data path: xT, weights, qT/kT/v, attention weights, output staging
#     all bf16 (PSUM accumulation stays f32). Halves DMA + SBUF traffic and
#     gives full PE rate at any moving width.
#   - x is pre-transposed on the host (parameter-packing style), removing all
#     on-chip x transposes (PE) and xT copies (DVE).
#   - DMA emission order: xT chunk 0 + wq/wk first, wv mid-stream, small
#     constants last, so the projection gate (full xT + wq/wk) clears ASAP.
#   - Startup: q/k head-block-0 cols [0:1024) accumulate per d-chunk as x
#     streams in; the bias rides the accumulation as a ones-row matmul so the
#     PSUM->SBUF moves are pure copies split across DVE and ACT in parallel.
#   - All remaining projections are "filler" units (q/k chunks split into two
#     4-d-chunk pieces) drained one per key-block iteration into the
#     attention phase, keeping PE busy while ACT grinds through exp.
#   - Attention per (head, sequence-half): scoresT[sk,sq] on PE, one exp per
#     key block on ACT (the only exp engine), multiplicative causal mask on
#     GPSIMD, PV accumulates outT+sums in PSUM via a ones column in v;
#     tail transposes back through PE and normalizes on DVE.

import numpy as np
import ml_dtypes

B, S, D, H = 2, 2048, 1024, 16
DK = D // H            # 64
NCORES = 8
HPC = 4                # heads per core
E = HPC * DK           # 256 output channels per core
EA = HPC * (DK + 1)    # 260 augmented v width (ones col per head)
P = 128
NSB = S // P           # 16 s-blocks
NDC = D // P           # 8 d-chunks
HALF = 1024

_cache = {}


def _build_module(reps=1):
    import concourse.bacc as bacc
    import concourse.mybir as mybir
    import concourse.tile as tile

    f32 = mybir.dt.float32
    bf16 = mybir.dt.bfloat16

    nc = bacc.Bacc("TRN2", target_bir_lowering=False, debug=False)

    xt_d = nc.dram_tensor("xt", [D, S], bf16, kind="ExternalInput")
    wq_d = nc.dram_tensor("wq_t", [D, E], bf16, kind="ExternalInput")
    wk_d = nc.dram_tensor("wk_t", [D, E], bf16, kind="ExternalInput")
    wv_d = nc.dram_tensor("wv_t", [D, EA], bf16, kind="ExternalInput")
    bq_d = nc.dram_tensor("bq", [1, E], f32, kind="ExternalInput")
    bk_d = nc.dram_tensor("bk", [1, E], f32, kind="ExternalInput")
    bqr_d = nc.dram_tensor("bqr", [1, E], bf16, kind="ExternalInput")
    bkr_d = nc.dram_tensor("bkr", [1, E], bf16, kind="ExternalInput")
    bv_d = nc.dram_tensor("bv", [1, EA], bf16, kind="ExternalInput")
    mask_d = nc.dram_tensor("mask", [P, P], bf16, kind="ExternalInput")
    ident_d = nc.dram_tensor("ident", [P, P], f32, kind="ExternalInput")
    ones_d = nc.dram_tensor("ones", [1, 512], bf16, kind="ExternalInput")
    out_d = nc.dram_tensor("out", [S, E], bf16, kind="ExternalOutput")

    with tile.TileContext(nc) as tc:
        for rep in range(reps):
            _emit_rep(nc, tc, rep, xt_d, wq_d, wk_d, wv_d, bq_d, bk_d, bv_d,
                      bqr_d, bkr_d, mask_d, ident_d, ones_d, out_d)

    nc.compile()
    return nc


def _emit_rep(nc, tc, rep, xt_d, wq_d, wk_d, wv_d, bq_d, bk_d, bv_d,
              bqr_d, bkr_d, mask_d, ident_d, ones_d, out_d):
    import concourse.mybir as mybir

    f32 = mybir.dt.float32
    bf16 = mybir.dt.bfloat16
    Exp = mybir.ActivationFunctionType.Exp

    with (
        tc.tile_pool(name=f"consts{rep}", bufs=1) as consts,
        tc.tile_pool(name=f"qkv{rep}", bufs=1) as qkv,
        tc.tile_pool(name=f"outst{rep}", bufs=1) as outst,
        tc.tile_pool(name=f"xt{rep}", bufs=1) as xtp,
        tc.tile_pool(name=f"pp{rep}", bufs=2, space="PSUM") as pp,
        tc.tile_pool(name=f"psc{rep}", bufs=2, space="PSUM") as pscp,
        tc.tile_pool(name=f"pacc{rep}", bufs=1, space="PSUM") as paccp,
        tc.tile_pool(name=f"attn{rep}", bufs=3) as attnp,
        tc.tile_pool(name=f"otn{rep}", bufs=2) as otnp,
    ):
        # ---- constants ----
        wq_sb = consts.tile([P, NDC, E], bf16, tag="wq")
        wk_sb = consts.tile([P, NDC, E], bf16, tag="wk")
        wv_sb = consts.tile([P, NDC, EA], bf16, tag="wv")
        bqc_sb = consts.tile([P, 2], f32, tag="bqc")
        bkc_sb = consts.tile([P, 2], f32, tag="bkc")
        bv_sb = consts.tile([1, EA], bf16, tag="bv")
        bqr_sb = consts.tile([1, E], bf16, tag="bqr")
        bkr_sb = consts.tile([1, E], bf16, tag="bkr")
        mask_sb = consts.tile([P, P], bf16, tag="mask")
        ident_sb = consts.tile([P, P], f32, tag="ident")
        ones_sb = consts.tile([1, 512], bf16, tag="ones")

        qT = qkv.tile([P, 2, S], bf16, tag="qT")
        kT = qkv.tile([P, 2, S], bf16, tag="kT")
        v_sb = qkv.tile([P, NSB, EA], bf16, tag="v")
        out_sb = outst.tile([P, NSB, E], bf16, tag="out")
        xT = xtp.tile([P, NDC, S], bf16, tag="xT")

        # ---- DMA order: xT first (with weights), small consts late ----
        def dma_x(dc):
            nc.sync.dma_start(out=xT[:, dc, :], in_=xt_d[dc * P:(dc + 1) * P, :])

        dma_x(0)
        nc.sync.dma_start(out=wq_sb, in_=wq_d[:].rearrange("(c p) e -> p c e", p=P))
        nc.sync.dma_start(out=wk_sb, in_=wk_d[:].rearrange("(c p) e -> p c e", p=P))
        nc.sync.dma_start(out=bqr_sb, in_=bqr_d[:])
        nc.sync.dma_start(out=bkr_sb, in_=bkr_d[:])
        nc.sync.dma_start(out=bv_sb, in_=bv_d[:])
        nc.sync.dma_start(out=ones_sb, in_=ones_d[:])
        for dc in (1, 2, 3):
            dma_x(dc)
        nc.sync.dma_start(out=wv_sb, in_=wv_d[:].rearrange("(c p) e -> p c e", p=P))
        for dc in (4, 5, 6, 7):
            dma_x(dc)
        nc.sync.dma_start(out=bqc_sb, in_=bq_d[:].rearrange("o (c p) -> p (o c)", p=P))
        nc.sync.dma_start(out=bkc_sb, in_=bk_d[:].rearrange("o (c p) -> p (o c)", p=P))
        nc.sync.dma_start(out=mask_sb, in_=mask_d[:])
        nc.sync.dma_start(out=ident_sb, in_=ident_d[:])

        # ---- startup: q/k eb0 [0:1024); bias rides as a ones-row matmul,
        # PSUM->SBUF copies split across DVE and ACT ----
        ps_q = pscp.tile([P, HALF], f32, tag="sc")
        ps_k = pscp.tile([P, HALF], f32, tag="sc")
        for dc in range(NDC):
            for sc in (0, 1):
                nc.tensor.matmul(
                    ps_q[:, sc * 512:(sc + 1) * 512],
                    lhsT=wq_sb[:, dc, 0:P],
                    rhs=xT[:, dc, sc * 512:(sc + 1) * 512],
                    start=(dc == 0), stop=False,
                )
                nc.tensor.matmul(
                    ps_k[:, sc * 512:(sc + 1) * 512],
                    lhsT=wk_sb[:, dc, 0:P],
                    rhs=xT[:, dc, sc * 512:(sc + 1) * 512],
                    start=(dc == 0), stop=False,
                )
        for sc in (0, 1):
            nc.tensor.matmul(
                ps_q[:, sc * 512:(sc + 1) * 512],
                lhsT=bqr_sb[0:1, 0:P], rhs=ones_sb[0:1, :],
                start=False, stop=True,
            )
            nc.tensor.matmul(
                ps_k[:, sc * 512:(sc + 1) * 512],
                lhsT=bkr_sb[0:1, 0:P], rhs=ones_sb[0:1, :],
                start=False, stop=True,
            )
        nc.vector.tensor_copy(qT[:, 0, 0:512], ps_q[:, 0:512])
        nc.scalar.copy(qT[:, 0, 512:HALF], ps_q[:, 512:HALF])
        nc.vector.tensor_copy(kT[:, 0, 0:512], ps_k[:, 0:512])
        nc.scalar.copy(kT[:, 0, 512:HALF], ps_k[:, 512:HALF])

        # ---- fillers: qk chunks as two 4-d-chunk pieces, v blocks whole ----
        qk_open = {}

        def emit_qk_part(which, eb, sc, part):
            w_sb = wq_sb if which == 0 else wk_sb
            bc = bqc_sb if which == 0 else bkc_sb
            dst = qT if which == 0 else kT
            key = (which, eb, sc)
            if part == 0:
                qk_open[key] = pp.tile(
                    [P, 512], f32, tag="pp",
                    name=f"qkps{which}_{eb}_{sc}")
            ps = qk_open[key]
            for dc in range(part * 4, part * 4 + 4):
                nc.tensor.matmul(
                    ps,
                    lhsT=w_sb[:, dc, eb * P:(eb + 1) * P],
                    rhs=xT[:, dc, sc * 512:(sc + 1) * 512],
                    start=(dc == 0),
                    stop=(dc == NDC - 1),
                )
            if part == 1:
                nc.vector.tensor_scalar_add(
                    dst[:, eb, sc * 512:(sc + 1) * 512], ps, bc[:, eb:eb + 1]
                )
                del qk_open[key]

        def emit_v_proj(sb):
            ps = pp.tile([P, 512], f32, tag="pp")
            pv = ps[:, :EA]
            for dc in range(NDC):
                nc.tensor.matmul(
                    pv, lhsT=xT[:, dc, sb * P:(sb + 1) * P], rhs=wv_sb[:, dc, :],
                    start=(dc == 0), stop=False,
                )
            nc.tensor.matmul(
                pv, lhsT=ones_sb[0:1, 0:P], rhs=bv_sb[0:1, :],
                start=False, stop=True,
            )
            nc.vector.tensor_copy(v_sb[:, sb, :], pv)

        filler = []
        for sb in range(0, 8):
            filler.append(lambda sb=sb: emit_v_proj(sb))
        for sc in (0, 1):
            for which in (0, 1):
                for part in (0, 1):
                    filler.append(lambda w=which, sc=sc, p=part: emit_qk_part(w, 1, sc, p))
        for sc in (2, 3):
            for which in (0, 1):
                for part in (0, 1):
                    filler.append(lambda w=which, sc=sc, p=part: emit_qk_part(w, 0, sc, p))
        for sb in range(8, NSB):
            filler.append(lambda sb=sb: emit_v_proj(sb))
        for sc in (2, 3):
            for which in (0, 1):
                for part in (0, 1):
                    filler.append(lambda w=which, sc=sc, p=part: emit_qk_part(w, 1, sc, p))

        def drain(n):
            for _ in range(n):
                if filler:
                    filler.pop(0)()

        # ---- attention ----
        def attn_head_half(h, half, jhook=None, per_block_dma=False):
            po = 64 * (h % 2)
            eb = h // 2
            kT_h = kT[po:po + DK, eb, :]
            qT_h = qT[po:po + DK, eb, :]
            lo = half * HALF
            hi = lo + HALF
            pacc = paccp.tile([65, HALF], f32, tag="pacc")

            def emit_pv(j, at):
                # PV pieces: absolute 512-aligned within [lo, hi)
                sb0 = max(j * P, lo)
                lhsT_v = v_sb[:, j, h * 65:(h + 1) * 65]
                m = sb0
                while m < hi:
                    w = min(512 - (m - lo) % 512, hi - m)
                    bank = (m - lo) // 512
                    j_last = min((lo + 512 * (bank + 1)) // P - 1, hi // P - 1)
                    nc.tensor.matmul(
                        pacc[:, m - lo:m - lo + w],
                        lhsT=lhsT_v,
                        rhs=at[:, m - sb0:m - sb0 + w],
                        start=(j == 0),
                        stop=(j == j_last),
                    )
                    m += w

            pending = None  # (j, at) whose PV is deferred one iteration
            for j in range(hi // P):
                ko = j * P
                sb0 = max(ko, lo)
                segw = hi - sb0
                ps = pscp.tile([P, HALF], f32, tag="sc")
                m = 0
                while m < segw:
                    w = min(512, segw - m)
                    nc.tensor.matmul(
                        ps[:, m:m + w],
                        lhsT=kT_h[:, ko:ko + P],
                        rhs=qT_h[:, sb0 + m:sb0 + m + w],
                        start=True,
                        stop=True,
                    )
                    m += w
                at = attnp.tile([P, HALF], bf16, tag="at")
                nc.scalar.activation(
                    out=at[:, :segw], in_=ps[:, :segw], func=Exp, scale=0.125
                )
                if ko >= lo:
                    nc.gpsimd.tensor_mul(at[:, 0:P], at[:, 0:P], mask_sb)
                if jhook is not None:
                    jhook(j)
                if pending is not None:
                    emit_pv(*pending)
                pending = (j, at)
            emit_pv(*pending)

            # tail: normalize + transpose back + stage
            otn = otnp.tile([65, HALF], f32, tag="otn")
            nc.vector.tensor_copy(otn, pacc)
            for il in range(HALF // P):
                i = half * 8 + il
                pot = pp.tile([P, 65], f32, tag="pp")
                nc.tensor.transpose(
                    pot, otn[:, il * P:(il + 1) * P], ident_sb[0:65, 0:65]
                )
                linv = otnp.tile([P, 1], f32, tag="linv")
                nc.vector.reciprocal(linv, pot[:, DK:DK + 1])
                nc.vector.tensor_scalar_mul(
                    out_sb[:, i, h * DK:(h + 1) * DK], pot[:, 0:DK], linv
                )
                if per_block_dma:
                    nc.sync.dma_start(
                        out=out_d[i * P:(i + 1) * P, :], in_=out_sb[:, i, :]
                    )

        hook = lambda j: drain(1)
        attn_head_half(0, 0, jhook=hook)
        attn_head_half(1, 0, jhook=hook)
        attn_head_half(2, 0, jhook=hook)
        attn_head_half(3, 0, jhook=hook)
        for i in range(8):
            nc.sync.dma_start(
                out=out_d[i * P:(i + 1) * P, :], in_=out_sb[:, i, :]
            )
        attn_head_half(0, 1, jhook=hook)
        attn_head_half(1, 1, jhook=hook)
        attn_head_half(2, 1, jhook=hook)
        attn_head_half(3, 1, jhook=hook, per_block_dma=True)


def _bf16(a):
    return np.asarray(a, dtype=ml_dtypes.bfloat16)


def _prep_core_inputs(inputs, c):
    x = np.asarray(inputs["x"], dtype=np.float32)
    b, hg = c // HPC, c % HPC
    e0 = hg * E

    wq = np.asarray(inputs["Wq"], dtype=np.float32)
    wk = np.asarray(inputs["Wk"], dtype=np.float32)
    wv = np.asarray(inputs["Wv"], dtype=np.float32)
    bq = np.asarray(inputs["bq"], dtype=np.float32)
    bk = np.asarray(inputs["bk"], dtype=np.float32)
    bv = np.asarray(inputs["bv"], dtype=np.float32)

    wq_t = np.ascontiguousarray(wq[e0:e0 + E, :].T)          # [D, E]
    wk_t = np.ascontiguousarray(wk[e0:e0 + E, :].T)
    wv_t = np.zeros((D, EA), dtype=np.float32)
    bv_a = np.zeros((1, EA), dtype=np.float32)
    for lh in range(HPC):
        cols = slice(lh * 65, lh * 65 + DK)
        rows = slice(e0 + lh * DK, e0 + lh * DK + DK)
        wv_t[:, cols] = wv[rows, :].T
        bv_a[0, cols] = bv[rows]
        bv_a[0, lh * 65 + DK] = 1.0                          # ones column

    mask = np.where(
        np.arange(P)[None, :] >= np.arange(P)[:, None], 1.0, 0.0
    ).astype(np.float32)

    return {
        "xt": _bf16(x[b].T),
        "wq_t": _bf16(wq_t),
        "wk_t": _bf16(wk_t),
        "wv_t": _bf16(wv_t),
        "bq": np.ascontiguousarray(bq[e0:e0 + E])[None, :],
        "bk": np.ascontiguousarray(bk[e0:e0 + E])[None, :],
        "bqr": _bf16(bq[e0:e0 + E][None, :]),
        "bkr": _bf16(bk[e0:e0 + E][None, :]),
        "bv": _bf16(bv_a),
        "mask": _bf16(mask),
        "ident": np.eye(P, dtype=np.float32),
        "ones": _bf16(np.ones((1, 512), dtype=np.float32)),
    }


def kernel(**inputs):
    from concourse.bass_utils import run_bass_kernel_spmd

    if "nc" not in _cache:
        _cache["nc"] = _build_module()
    nc = _cache["nc"]

    in_maps = [_prep_core_inputs(inputs, c) for c in range(NCORES)]
    res = run_bass_kernel_spmd(nc, in_maps, core_ids=list(range(NCORES)))

    out = np.empty((B, S, D), dtype=np.float32)
    for c in range(NCORES):
        b, hg = c // HPC, c % HPC
        out[b, :, hg * E:(hg + 1) * E] = res.results[c]["out"].astype(np.float32)
    return out
